# revision 18
# baseline (speedup 1.0000x reference)
"""HAGMoE Trainium2 kernel: hierarchical-routed 24-expert MoE, expert-parallel on 8 cores.

Reference computation (B=1024, H=768, I=3072, G=3 groups, E=8 experts/group):
    h_cond  = cat(h_fused, h_aspect) @ Wc + bc
    p_group = softmax(h_fused @ Wg + bg)
    p_exp   = softmax(h_cond @ Wr[g] + br[g])  per group
    h_moe   = sum_{g,e} p_group[:,g] * p_exp[:,g,e] * fc2(gelu(fc1(h_fused)))
    out     = h_fused + h_moe
Sharding: core c owns experts (g, c) for g=0..2 (one expert per group).  The
cond-proj is folded through the expert routers on the host (Wcr = Wc @ Wr), and
within-group expert columns are permuted per core so every core's experts sit at
logit columns {0, 8, 16} -> identical SPMD program, per-core weight data only.
All matmuls (experts AND routing) run in fp8e4 DoubleRow mode (2x PE
throughput, fp32 PSUM accumulate) with host-side scaling to dodge fp8
subnormals; the exp() activation absorbs the routing descale.  A second
compiled variant handles nonzero fc2 bias (b2) via a DVE bias path; the
common b2==0 case skips that work entirely.  Host gathers:
out = h_fused + sum_c partial_c.
"""

import os
import sys

if "/opt/trn_rl_repo" not in sys.path:
    sys.path.insert(0, "/opt/trn_rl_repo")

import numpy as np
import ml_dtypes

B, H, I, G, E = 1024, 768, 3072, 3, 8
NCORES = 8
BF16 = ml_dtypes.bfloat16
FP8 = ml_dtypes.float8_e4m3

SX = 16.0    # h_fused / h_aspect scale before fp8 cast
SRF = 128.0  # router weight scale before fp8 cast
SW1 = 64.0   # W1 scale before fp8 cast
SW2 = 64.0   # W2 scale before fp8 cast

_nc_cache = {}


def _build_nc(with_b2):
    from concourse import bacc
    import concourse.mybir as mybir
    from concourse.tile import TileContext

    dt = mybir.dt
    AF = mybir.ActivationFunctionType
    DR = mybir.MatmulPerfMode.DoubleRow

    nc = bacc.Bacc("TRN2", target_bir_lowering=False, debug=False, num_devices=NCORES)

    NR = 32  # logit cols (24 expert + 3 group + 5 pad: dual-fp8 ldweights needs even stride)

    # ---- DRAM I/O ----
    # all inputs pre-packed host-side into SBUF layout: one long contiguous
    # run per partition -> minimal DMA descriptor count
    KHc = H // 128
    KIc = I // 128
    MIc = I // 128
    xt8_d = nc.dram_tensor("xt8", [128, KHc * B], dt.float8e4, kind="ExternalInput")
    xa8_d = nc.dram_tensor("xa8", [128, KHc * B], dt.float8e4, kind="ExternalInput")
    rf_d = nc.dram_tensor("rf", [128, KHc * NR], dt.float8e4, kind="ExternalInput")
    ra_d = nc.dram_tensor("ra", [128, KHc * NR], dt.float8e4, kind="ExternalInput")
    bcat_d = nc.dram_tensor("bcat", [NR, 1], dt.float32, kind="ExternalInput")
    # w1 packed m-major: [p][(m k i128)]; w2 packed k-major: [p][(k h)]
    w1_d = nc.dram_tensor("w1", [G, 128, MIc * KHc * 128], dt.float8e4, kind="ExternalInput")
    b1_d = nc.dram_tensor("b1", [G, 128, MIc], dt.float32, kind="ExternalInput")
    w2_d = nc.dram_tensor("w2", [G, 128, KIc * H], dt.float8e4, kind="ExternalInput")
    if with_b2:
        b2_d = nc.dram_tensor("b2", [G, H], dt.bfloat16, kind="ExternalInput")
    out_d = nc.dram_tensor("out", [B, H], dt.bfloat16, kind="ExternalOutput")

    KH = H // 128   # 6 k-chunks for the H contraction
    KI = I // 128   # 24 k-chunks for the I contraction
    MB = B // 128   # 8 token chunks
    MI = I // 128   # 24 i chunks (fc1 output partitions)

    from concourse.masks import make_identity

    with TileContext(nc) as tc:
        with (
            tc.tile_pool(name="x8p", bufs=1) as x8p,
            tc.tile_pool(name="h1gp", bufs=2) as h1gp,
            tc.tile_pool(name="accp", bufs=1) as accp,
            tc.tile_pool(name="wp", bufs=2) as wp,
            tc.tile_pool(name="constp", bufs=1) as constp,
            tc.tile_pool(name="b1p", bufs=2) as b1p,
            tc.tile_pool(name="wselp", bufs=1) as wselp,
            tc.tile_pool(name="tmpp", bufs=4) as tmpp,
            tc.tile_pool(name="smp", bufs=8) as smp,
            tc.tile_pool(name="routp", bufs=1) as routp,
        ):
            # ---- DMA issue order tracks the tensor queue's needs ----
            # fc1(0) runs first: first slices of w1(0) + x8 lead everything
            W1COLS = MI * KH * 128
            w1t0 = wp.tile([128, W1COLS], dt.float8e4, name="w1t0", tag="w1")
            w1v0 = w1t0[:].rearrange("p (m k i) -> p m k i", k=KH, i=128)
            NP = 6  # w1t0 DMA pieces (m-chunks arrive in consumption order)
            MPP = MI // NP  # m-chunks per piece
            CPP = W1COLS // NP
            nc.sync.dma_start(
                out=w1t0[:, 0:CPP], in_=w1_d.ap()[0, :, 0:CPP]
            )
            x8 = x8p.tile([128, KH * B], dt.float8e4, name="x8t")
            x8v = x8[:].rearrange("p (k b) -> p k b", b=B)
            for kk in range(KH // 2):
                nc.sync.dma_start(
                    out=x8[:, kk * 2 * B : (kk + 1) * 2 * B],
                    in_=xt8_d.ap()[:, kk * 2 * B : (kk + 1) * 2 * B],
                )
            nc.sync.dma_start(
                out=w1t0[:, CPP : 2 * CPP], in_=w1_d.ap()[0, :, CPP : 2 * CPP]
            )
            b1t0 = b1p.tile([128, MI], dt.float32, name="b1t0", tag="b1")
            nc.sync.dma_start(out=b1t0[:], in_=b1_d.ap()[0])
            # routing inputs
            xa8 = routp.tile([128, KH * B], dt.float8e4, name="xa8t")
            nc.sync.dma_start(out=xa8[:], in_=xa8_d.ap())
            rfb = routp.tile([128, KH * NR], dt.float8e4, name="rfbt")
            nc.sync.dma_start(out=rfb[:], in_=rf_d.ap())
            rab = routp.tile([128, KH * NR], dt.float8e4, name="rabt")
            nc.sync.dma_start(out=rab[:], in_=ra_d.ap())
            bcatT = routp.tile([NR, 1], dt.float32, name="bcatTt")
            nc.sync.dma_start(out=bcatT[:], in_=bcat_d.ap())
            # rest of w1(0)
            for piece in range(2, NP):
                io = piece * CPP
                nc.sync.dma_start(
                    out=w1t0[:, io : io + CPP], in_=w1_d.ap()[0, :, io : io + CPP]
                )
            if with_b2:
                # b2 replicated across partitions (for the DVE bias path)
                b2repl = constp.tile([128, G * H], dt.bfloat16, name="b2replt")
                nc.sync.dma_start(
                    out=b2repl[:],
                    in_=b2_d.ap()
                    .rearrange("g h -> () (g h)")
                    .broadcast_to([128, G * H]),
                )
            w2t0 = wp.tile([128, KI * H], dt.float8e4, name="w2t0", tag="w2")
            nc.sync.dma_start(out=w2t0[:], in_=w2_d.ap()[0])

            acc = accp.tile([128, MB * H], dt.float32, name="acct")
            accb = accp.tile([128, MB * H], dt.bfloat16, name="accbt")
            wsel = wselp.tile([128, MB * G], dt.float32, name="wselt")
            ident = constp.tile([32, 32], dt.float32, name="identt")
            make_identity(nc, ident[:])
            wrhs = constp.tile([32, 512], dt.float32, name="wrhst")
            nc.vector.memset(wrhs[:], 0.0)

            xa8v = xa8[:].rearrange("p (k b) -> p k b", b=B)
            rfv = rfb[:].rearrange("p (k n) -> p k n", n=NR)
            rav = rab[:].rearrange("p (k n) -> p k n", n=NR)

            def emit_fc1(j, w1v, b1t, h1g, ps1, m_range):
                for m in m_range:
                    psAB = ps1.tile(
                        [128, 1024], dt.float32, name=f"psAB{j}_{m}", tag="ps1t"
                    )
                    for k in range(KH // 2):
                        lhs = w1v[:, m, 2 * k : 2 * k + 2, :]
                        nc.tensor.matmul(
                            psAB[:, 0:512],
                            lhs,
                            x8v[:, 2 * k : 2 * k + 2, 0:512],
                            start=(k == 0),
                            stop=(k == KH // 2 - 1),
                            perf_mode=DR,
                        )
                        nc.tensor.matmul(
                            psAB[:, 512:1024],
                            lhs,
                            x8v[:, 2 * k : 2 * k + 2, 512:1024],
                            start=(k == 0),
                            stop=(k == KH // 2 - 1),
                            perf_mode=DR,
                        )
                    if m == MI - 1:
                        for c0 in (0, 512):
                            nc.scalar.activation(
                                h1g[:, m * B + c0 : m * B + c0 + 512],
                                psAB[:, c0 : c0 + 512],
                                AF.Gelu,
                                bias=b1t[:, m : m + 1],
                                scale=1.0 / (SX * SW1),
                            )
                    else:
                        nc.scalar.activation(
                            h1g[:, m * B : (m + 1) * B],
                            psAB[:],
                            AF.Gelu,
                            bias=b1t[:, m : m + 1],
                            scale=1.0 / (SX * SW1),
                        )

            def emit_fc2(j, w2t, h1g, ps2):
                w2v = w2t[:].rearrange("p (k h) -> p k h", h=H)
                h1v = h1g[:].rearrange("p (m b) -> p m b", b=B)
                for t in range(MB):
                    p2 = ps2.tile([128, 1024], dt.float32, name=f"p2{j}_{t}", tag="p2")
                    for k in range(KI // 2):
                        lhs = h1v[:, 2 * k : 2 * k + 2, t * 128 : (t + 1) * 128]
                        nc.tensor.matmul(
                            p2[:, 0:512],
                            lhs,
                            w2v[:, 2 * k : 2 * k + 2, 0:512],
                            start=(k == 0),
                            stop=(k == KI // 2 - 1),
                            perf_mode=DR,
                        )
                        nc.tensor.matmul(
                            p2[:, 512:768],
                            lhs,
                            w2v[:, 2 * k : 2 * k + 2, 512:768],
                            start=(k == 0),
                            stop=(k == KI // 2 - 1),
                            perf_mode=DR,
                        )
                    # weighted accumulate into acc (wsel carries the 1/SW2):
                    # ScalarE does the p_sel multiply, DVE the accumulate
                    wcol = wsel[:, t * G + j : t * G + j + 1]
                    # combine runs fully on DVE: ScalarE stays free for the
                    # gelu ACTs, which otherwise head-of-line-block the psum
                    # drain at expert boundaries
                    if j == 0 and not with_b2:
                        nc.vector.tensor_scalar_mul(
                            acc[:, t * H : (t + 1) * H], p2[:, 0:768], wcol[:]
                        )
                    elif j < G - 1:
                        tmpc = tmpp.tile(
                            [128, H], dt.float32, name=f"tmpc{j}_{t}", tag="tmpc"
                        )
                        nc.vector.tensor_scalar_mul(tmpc[:], p2[:, 0:768], wcol[:])
                        nc.vector.tensor_add(
                            acc[:, t * H : (t + 1) * H],
                            acc[:, t * H : (t + 1) * H],
                            tmpc[:],
                        )
                    else:
                        # final expert: both p2-reading muls first (frees the
                        # psum bank for t+2 asap), then the adds + bf16 DMAs
                        tcs = []
                        for c0, c1 in ((0, 512), (512, 768)):
                            tmpc = tmpp.tile(
                                [128, c1 - c0],
                                dt.float32,
                                name=f"tmpc{j}_{t}_{c0}",
                                tag=f"tmpch{c0}",
                            )
                            nc.vector.tensor_scalar_mul(tmpc[:], p2[:, c0:c1], wcol[:])
                            tcs.append((c0, c1, tmpc))
                        for c0, c1, tmpc in tcs:
                            nc.vector.tensor_add(
                                accb[:, t * H + c0 : t * H + c1],
                                acc[:, t * H + c0 : t * H + c1],
                                tmpc[:],
                            )
                            nc.sync.dma_start(
                                out=out_d.ap()[t * 128 : (t + 1) * 128, c0:c1],
                                in_=accb[:, t * H + c0 : t * H + c1],
                            )

            with tc.tile_pool(name="ps1", bufs=2, space="PSUM") as ps1:
                h1g0 = h1gp.tile([128, MI * B], dt.float8e4, name="h1g0", tag="h1g")
                expT = routp.tile([NR, B], dt.float32, name="expTt")

                with (
                    tc.tile_pool(name="psT", bufs=1, space="PSUM") as psTp,
                    tc.tile_pool(name="psm", bufs=2, space="PSUM") as psmp,
                ):
                    psT = psTp.tile([NR, B], dt.float32, name="psTt")
                    # PE p-state warmup while the input DMAs stream
                    for w in range(2):
                        nc.tensor.matmul(
                            psT[0:32, 0:512],
                            ident[:],
                            wrhs[:],
                            start=True,
                            stop=True,
                        )

                    # fc1(0) m0-3 leads (first w1 piece); routing matmuls slot
                    # in behind, then the rest of fc1(0)
                    emit_fc1(0, w1v0, b1t0, h1g0, ps1, range(0, MPP))

                    # routing: logitsT[NR, B] in fp8 DoubleRow, scale absorbed
                    # by the exp() activation
                    for k in range(KH // 2):
                        for n in range(2):
                            nc.tensor.matmul(
                                psT[:, n * 512 : (n + 1) * 512],
                                rfv[:, 2 * k : 2 * k + 2, :],
                                x8v[:, 2 * k : 2 * k + 2, n * 512 : (n + 1) * 512],
                                start=(k == 0),
                                stop=False,
                                perf_mode=DR,
                            )
                    for k in range(KH // 2):
                        for n in range(2):
                            nc.tensor.matmul(
                                psT[:, n * 512 : (n + 1) * 512],
                                rav[:, 2 * k : 2 * k + 2, :],
                                xa8v[:, 2 * k : 2 * k + 2, n * 512 : (n + 1) * 512],
                                start=False,
                                stop=(k == KH // 2 - 1),
                                perf_mode=DR,
                            )
                    emit_fc1(0, w1v0, b1t0, h1g0, ps1, range(MPP, MI))

                    # exp(logits + bias) in one ACT op (small logits: no
                    # max-subtract); 1/2048 descale folded into the ACT.
                    # Emitted after fc1(0) so it doesn't head-of-line-block
                    # the gelu ACTs that gate fc1's psum recycling.
                    nc.scalar.activation(
                        expT[:], psT[:], AF.Exp, bias=bcatT[:], scale=1.0 / (SX * SRF)
                    )

                    # transpose expT to token-major (one psum tile), then a
                    # batched DVE softmax tail over all 8 token chunks
                    trp8 = psmp.tile([128, MB * NR], dt.float32, name="trp8", tag="trp")
                    for m in range(MB):
                        nc.tensor.transpose(
                            trp8[:, m * NR : (m + 1) * NR],
                            expT[:, m * 128 : (m + 1) * 128],
                            ident[0:NR, 0:NR],
                        )
                    t8 = trp8[:].rearrange("p (m n) -> p m n", n=NR)
                    sg8 = smp.tile([128, MB], dt.float32, name="sg8")
                    nc.vector.reduce_sum(sg8[:], t8[:, :, 24:27], axis=mybir.AxisListType.X)
                    rg8 = smp.tile([128, MB], dt.float32, name="rg8")
                    nc.vector.reciprocal(rg8[:], sg8[:])
                    pgn8 = smp.tile([128, MB * G], dt.float32, name="pgn8")
                    pgn8v = pgn8[:].rearrange("p (m g) -> p m g", g=G)
                    nc.vector.tensor_mul(
                        pgn8v,
                        t8[:, :, 24:27],
                        rg8[:].rearrange("p m -> p m ()").broadcast_to([128, MB, G]),
                    )
                    se8 = smp.tile([128, MB * G], dt.float32, name="se8")
                    se8v = se8[:].rearrange("p (m g) -> p m g", g=G)
                    nc.vector.reduce_sum(
                        se8v,
                        t8[:, :, 0:24].rearrange("p m (g e) -> p m g e", e=E),
                        axis=mybir.AxisListType.X,
                    )
                    re8 = smp.tile([128, MB * G], dt.float32, name="re8")
                    nc.vector.reciprocal(re8[:], se8[:])
                    pe08 = smp.tile([128, MB * G], dt.float32, name="pe08")
                    nc.vector.tensor_mul(
                        pe08[:].rearrange("p (m g) -> p m g", g=G),
                        t8[:, :, 0:24].rearrange("p m (g e) -> p m g e", e=E)[:, :, :, 0],
                        re8[:].rearrange("p (m g) -> p m g", g=G),
                    )
                    # wsel = p_group * p_expert0 / SW2 (fc2 descale folded in)
                    nc.vector.tensor_mul(wsel[:], pe08[:], pgn8[:])
                    nc.vector.tensor_scalar_mul(wsel[:], wsel[:], 1.0 / SW2)

                    for m in range(MB):
                        if with_b2:
                            # acc[t] bias init: sum_j wsel[t,j]*(SW2*b2[j,:])
                            bt = tmpp.tile(
                                [128, H], dt.float32, name=f"bt{m}", tag="tmpc"
                            )
                            nc.vector.tensor_scalar_mul(
                                acc[:, m * H : (m + 1) * H],
                                b2repl[:, 0:H],
                                wsel[:, m * G : m * G + 1],
                            )
                            for j in range(1, G):
                                nc.vector.tensor_scalar_mul(
                                    bt[:],
                                    b2repl[:, j * H : (j + 1) * H],
                                    wsel[:, m * G + j : m * G + j + 1],
                                )
                                nc.vector.tensor_add(
                                    acc[:, m * H : (m + 1) * H],
                                    acc[:, m * H : (m + 1) * H],
                                    bt[:],
                                )

                # ---- fc2(0) + experts 1,2 ----
                with tc.tile_pool(name="ps2", bufs=2, space="PSUM") as ps2:
                    emit_fc2(0, w2t0, h1g0, ps2)
                    for j in range(1, G):
                        w1t = wp.tile([128, W1COLS], dt.float8e4, name=f"w1t{j}", tag="w1")
                        nc.sync.dma_start(out=w1t[:], in_=w1_d.ap()[j])
                        b1t = b1p.tile([128, MI], dt.float32, name=f"b1t{j}", tag="b1")
                        nc.sync.dma_start(out=b1t[:], in_=b1_d.ap()[j])
                        w2t = wp.tile([128, KI * H], dt.float8e4, name=f"w2t{j}", tag="w2")
                        nc.sync.dma_start(out=w2t[:], in_=w2_d.ap()[j])
                        h1g = h1gp.tile(
                            [128, MI * B], dt.float8e4, name=f"h1g{j}", tag="h1g"
                        )
                        w1v = w1t[:].rearrange("p (m k i) -> p m k i", k=KH, i=128)
                        # m22/23 first: fc2 t0's k=11 needs their gelu output,
                        # so get their ACTs off the critical path early
                        emit_fc1(j, w1v, b1t, h1g, ps1, [MI - 2, MI - 1] + list(range(MI - 2)))
                        emit_fc2(j, w2t, h1g, ps2)

    nc.compile()
    return nc


def _get_nc(with_b2=False):
    if with_b2 not in _nc_cache:
        _nc_cache[with_b2] = _build_nc(with_b2)
    return _nc_cache[with_b2]


def _prepare(inputs):
    h_fused = np.asarray(inputs["h_fused"], np.float32)
    h_aspect = np.asarray(inputs["h_aspect"], np.float32)
    Wc = np.asarray(inputs["Wc"], np.float32)
    bc = np.asarray(inputs["bc"], np.float32)
    Wg = np.asarray(inputs["Wg"], np.float32)
    bg = np.asarray(inputs["bg"], np.float32)
    Wr = np.asarray(inputs["Wr"], np.float32)
    br = np.asarray(inputs["br"], np.float32)
    W1 = np.asarray(inputs["W1"], np.float32)
    b1 = np.asarray(inputs["b1"], np.float32)
    W2 = np.asarray(inputs["W2"], np.float32)
    b2 = np.asarray(inputs["b2"], np.float32)

    with_b2 = bool(np.any(b2))

    def q8(x, s):
        return np.clip(np.asarray(x, np.float64) * s, -240.0, 240.0).astype(FP8)

    # fold cond_proj through the expert routers (float64 for the fold)
    Wcr = np.einsum("ch,ghe->cge", Wc.astype(np.float64), Wr.astype(np.float64))
    bcr = np.einsum("h,ghe->ge", bc.astype(np.float64), Wr.astype(np.float64)) + br

    KH, KI, MI = H // 128, I // 128, I // 128

    def pack_pk(a, ncols):
        # [K*128, ncols] -> [128, K*ncols] partition-major
        k = a.shape[0] // 128
        return np.ascontiguousarray(
            a.reshape(k, 128, ncols).transpose(1, 0, 2).reshape(128, k * ncols)
        )

    xt8 = pack_pk(q8(h_fused.T, SX), B)
    xa8 = pack_pk(q8(h_aspect.T, SX), B)

    in_maps = []
    for c in range(NCORES):
        perm = [c] + [e for e in range(E) if e != c]
        Wcr_p = Wcr[:, :, perm]  # [2H, G, E]
        bcr_p = np.asarray(bcr, np.float64)[:, perm]  # [G, E]
        rf = np.concatenate(
            [Wcr_p[:H].reshape(H, G * E), Wg.astype(np.float64),
             np.zeros((H, 5), np.float64)], axis=1
        )
        ra = np.concatenate(
            [Wcr_p[H:].reshape(H, G * E), np.zeros((H, G + 5), np.float64)], axis=1
        )
        bcat = np.concatenate(
            [bcr_p.reshape(G * E), bg.astype(np.float64), np.zeros(5)]
        )[:, None].astype(np.float32)
        # w1[g]: [H, I] -> [128, (m k i128)] m-major; w2[g]: [I, H] -> [128, (k h)]
        w1p = np.stack(
            [
                q8(W1[g, c], SW1)
                .reshape(KH, 128, MI, 128)
                .transpose(1, 2, 0, 3)
                .reshape(128, MI * KH * 128)
                for g in range(G)
            ]
        )
        w2p = np.stack([pack_pk(q8(W2[g, c], SW2), H) for g in range(G)])
        b1p_ = np.stack(
            [np.ascontiguousarray(b1[g, c].reshape(MI, 128).T) for g in range(G)]
        )
        im = {
            "xt8": xt8,
            "xa8": xa8,
            "rf": pack_pk(q8(rf, SRF), 32),
            "ra": pack_pk(q8(ra, SRF), 32),
            "bcat": np.ascontiguousarray(bcat),
            "w1": np.ascontiguousarray(w1p),
            "b1": np.ascontiguousarray(b1p_),
            "w2": np.ascontiguousarray(w2p),
        }
        if with_b2:
            im["b2"] = (np.ascontiguousarray(b2[:, c]) * SW2).astype(BF16)
        in_maps.append(im)

    return h_fused, in_maps, with_b2


def kernel(**inputs):
    from concourse.bass_utils import run_bass_kernel_spmd

    h_fused, in_maps, with_b2 = _prepare(inputs)
    nc = _get_nc(with_b2)
    res = run_bass_kernel_spmd(nc, in_maps, core_ids=list(range(NCORES)))
    out = h_fused.copy()
    for c in range(NCORES):
        out += np.asarray(res.results[c]["out"], np.float32)
    return out


def run_traced(**inputs):
    """Profiled run: returns BassKernelResults with exec_time_ns."""
    from concourse.bass_utils import run_bass_kernel_spmd

    h_fused, in_maps, with_b2 = _prepare(inputs)
    nc = _get_nc(with_b2)
    res = run_bass_kernel_spmd(nc, in_maps, core_ids=list(range(NCORES)), trace=True)
    return res


# revision 19
# speedup vs baseline: 1.1838x; 1.1838x over previous
"""HAGMoE Trainium2 kernel: hierarchical-routed 24-expert MoE, expert-parallel on 8 cores.

Reference computation (B=1024, H=768, I=3072, G=3 groups, E=8 experts/group):
    h_cond  = cat(h_fused, h_aspect) @ Wc + bc
    p_group = softmax(h_fused @ Wg + bg)
    p_exp   = softmax(h_cond @ Wr[g] + br[g])  per group
    h_moe   = sum_{g,e} p_group[:,g] * p_exp[:,g,e] * fc2(gelu(fc1(h_fused)))
    out     = h_fused + h_moe
Sharding: core c owns experts (g, c) for g=0..2 (one expert per group).  The
cond-proj is folded through the expert routers on the host (Wcr = Wc @ Wr), and
within-group expert columns are permuted per core so every core's experts sit at
logit columns {0, 8, 16} -> identical SPMD program, per-core weight data only.
All matmuls (experts AND routing) run in fp8e4 DoubleRow mode (2x PE
throughput, fp32 PSUM accumulate) with host-side scaling to dodge fp8
subnormals; the exp() activation absorbs the routing descale.  A second
compiled variant handles nonzero fc2 bias (b2) via a DVE bias path; the
common b2==0 case skips that work entirely.  Host gathers:
out = h_fused + sum_c partial_c.
"""

import os
import sys

if "/opt/trn_rl_repo" not in sys.path:
    sys.path.insert(0, "/opt/trn_rl_repo")

import numpy as np
import ml_dtypes

B, H, I, G, E = 1024, 768, 3072, 3, 8
NCORES = 8
BF16 = ml_dtypes.bfloat16
FP8 = ml_dtypes.float8_e4m3

SX = 16.0    # h_fused / h_aspect scale before fp8 cast
SRF = 128.0  # router weight scale before fp8 cast
SW1 = 64.0   # W1 scale before fp8 cast
SW2 = 64.0   # W2 scale before fp8 cast

_nc_cache = {}


def _build_nc(with_b2):
    from concourse import bacc
    import concourse.mybir as mybir
    from concourse.tile import TileContext

    dt = mybir.dt
    AF = mybir.ActivationFunctionType
    DR = mybir.MatmulPerfMode.DoubleRow

    nc = bacc.Bacc("TRN2", target_bir_lowering=False, debug=False, num_devices=NCORES)

    NR = 32  # logit cols (24 expert + 3 group + 5 pad: dual-fp8 ldweights needs even stride)

    # ---- DRAM I/O ----
    # all inputs pre-packed host-side into SBUF layout: one long contiguous
    # run per partition -> minimal DMA descriptor count
    KHc = H // 128
    KIc = I // 128
    MIc = I // 128
    xt8_d = nc.dram_tensor("xt8", [128, KHc * B], dt.float8e4, kind="ExternalInput")
    xa8_d = nc.dram_tensor("xa8", [128, KHc * B], dt.float8e4, kind="ExternalInput")
    rf_d = nc.dram_tensor("rf", [128, KHc * NR], dt.float8e4, kind="ExternalInput")
    ra_d = nc.dram_tensor("ra", [128, KHc * NR], dt.float8e4, kind="ExternalInput")
    bcat_d = nc.dram_tensor("bcat", [NR, 1], dt.float32, kind="ExternalInput")
    # w1 packed m-major: [p][(m k i128)]; w2 packed k-major: [p][(k h)]
    w1_d = nc.dram_tensor("w1", [G, 128, MIc * KHc * 128], dt.float8e4, kind="ExternalInput")
    b1_d = nc.dram_tensor("b1", [G, 128, MIc], dt.float32, kind="ExternalInput")
    w2_d = nc.dram_tensor("w2", [G, 128, KIc * H], dt.float8e4, kind="ExternalInput")
    if with_b2:
        b2_d = nc.dram_tensor("b2", [G, H], dt.bfloat16, kind="ExternalInput")
    out_d = nc.dram_tensor("out", [B, H], dt.bfloat16, kind="ExternalOutput")

    KH = H // 128   # 6 k-chunks for the H contraction
    KI = I // 128   # 24 k-chunks for the I contraction
    MB = B // 128   # 8 token chunks
    MI = I // 128   # 24 i chunks (fc1 output partitions)

    from concourse.masks import make_identity

    with TileContext(nc) as tc:
        with (
            tc.tile_pool(name="x8p", bufs=1) as x8p,
            tc.tile_pool(name="h1gp", bufs=2) as h1gp,
            tc.tile_pool(name="accp", bufs=1) as accp,
            tc.tile_pool(name="wp", bufs=2) as wp,
            tc.tile_pool(name="constp", bufs=1) as constp,
            tc.tile_pool(name="b1p", bufs=2) as b1p,
            tc.tile_pool(name="wselp", bufs=1) as wselp,
            tc.tile_pool(name="tmpp", bufs=4) as tmpp,
            tc.tile_pool(name="smp", bufs=8) as smp,
            tc.tile_pool(name="routp", bufs=1) as routp,
        ):
            # ---- DMA issue order tracks the tensor queue's needs ----
            # fc1(0) runs first: first slices of w1(0) + x8 lead everything
            W1COLS = MI * KH * 128
            w1t0 = wp.tile([128, W1COLS], dt.float8e4, name="w1t0", tag="w1")
            w1v0 = w1t0[:].rearrange("p (m k i) -> p m k i", k=KH, i=128)
            NP = 6  # w1t0 DMA pieces (m-chunks arrive in consumption order)
            MPP = MI // NP  # m-chunks per piece
            CPP = W1COLS // NP
            nc.sync.dma_start(
                out=w1t0[:, 0:CPP], in_=w1_d.ap()[0, :, 0:CPP]
            )
            x8 = x8p.tile([128, KH * B], dt.float8e4, name="x8t")
            x8v = x8[:].rearrange("p (k b) -> p k b", b=B)
            for kk in range(KH // 2):
                nc.sync.dma_start(
                    out=x8[:, kk * 2 * B : (kk + 1) * 2 * B],
                    in_=xt8_d.ap()[:, kk * 2 * B : (kk + 1) * 2 * B],
                )
            nc.sync.dma_start(
                out=w1t0[:, CPP : 2 * CPP], in_=w1_d.ap()[0, :, CPP : 2 * CPP]
            )
            b1t0 = b1p.tile([128, MI], dt.float32, name="b1t0", tag="b1")
            nc.sync.dma_start(out=b1t0[:], in_=b1_d.ap()[0])
            # routing inputs
            xa8 = routp.tile([128, KH * B], dt.float8e4, name="xa8t")
            nc.sync.dma_start(out=xa8[:], in_=xa8_d.ap())
            rfb = routp.tile([128, KH * NR], dt.float8e4, name="rfbt")
            nc.sync.dma_start(out=rfb[:], in_=rf_d.ap())
            rab = routp.tile([128, KH * NR], dt.float8e4, name="rabt")
            nc.sync.dma_start(out=rab[:], in_=ra_d.ap())
            bcatT = routp.tile([NR, 1], dt.float32, name="bcatTt")
            nc.sync.dma_start(out=bcatT[:], in_=bcat_d.ap())
            # rest of w1(0)
            for piece in range(2, NP):
                io = piece * CPP
                nc.sync.dma_start(
                    out=w1t0[:, io : io + CPP], in_=w1_d.ap()[0, :, io : io + CPP]
                )
            if with_b2:
                # b2 replicated across partitions (for the DVE bias path)
                b2repl = constp.tile([128, G * H], dt.bfloat16, name="b2replt")
                nc.sync.dma_start(
                    out=b2repl[:],
                    in_=b2_d.ap()
                    .rearrange("g h -> () (g h)")
                    .broadcast_to([128, G * H]),
                )
            w2t0 = wp.tile([128, KI * H], dt.float8e4, name="w2t0", tag="w2")
            nc.sync.dma_start(out=w2t0[:], in_=w2_d.ap()[0])

            acc = accp.tile([128, MB * H], dt.float32, name="acct")
            accb = accp.tile([128, MB * H], dt.bfloat16, name="accbt")
            wsel = wselp.tile([128, MB * G], dt.float32, name="wselt")
            ident = constp.tile([32, 32], dt.float32, name="identt")
            make_identity(nc, ident[:])
            wrhs = constp.tile([32, 512], dt.float32, name="wrhst")
            nc.vector.memset(wrhs[:], 0.0)

            xa8v = xa8[:].rearrange("p (k b) -> p k b", b=B)
            rfv = rfb[:].rearrange("p (k n) -> p k n", n=NR)
            rav = rab[:].rearrange("p (k n) -> p k n", n=NR)

            def emit_fc1(j, w1v, b1t, h1g, ps1, m_range):
                for m in m_range:
                    psAB = ps1.tile(
                        [128, 1024], dt.float32, name=f"psAB{j}_{m}", tag="ps1t"
                    )
                    for k in range(KH // 2):
                        lhs = w1v[:, m, 2 * k : 2 * k + 2, :]
                        nc.tensor.matmul(
                            psAB[:, 0:512],
                            lhs,
                            x8v[:, 2 * k : 2 * k + 2, 0:512],
                            start=(k == 0),
                            stop=(k == KH // 2 - 1),
                            perf_mode=DR,
                        )
                        nc.tensor.matmul(
                            psAB[:, 512:1024],
                            lhs,
                            x8v[:, 2 * k : 2 * k + 2, 512:1024],
                            start=(k == 0),
                            stop=(k == KH // 2 - 1),
                            perf_mode=DR,
                        )
                    if m == MI - 1:
                        for c0 in (0, 512):
                            nc.scalar.activation(
                                h1g[:, m * B + c0 : m * B + c0 + 512],
                                psAB[:, c0 : c0 + 512],
                                AF.Gelu,
                                bias=b1t[:, m : m + 1],
                                scale=1.0 / (SX * SW1),
                            )
                    else:
                        nc.scalar.activation(
                            h1g[:, m * B : (m + 1) * B],
                            psAB[:],
                            AF.Gelu,
                            bias=b1t[:, m : m + 1],
                            scale=1.0 / (SX * SW1),
                        )

            def emit_fc2(j, w2t, h1g, ps2):
                w2v = w2t[:].rearrange("p (k h) -> p k h", h=H)
                h1v = h1g[:].rearrange("p (m b) -> p m b", b=B)
                for t in range(MB):
                    p2 = ps2.tile([128, 1024], dt.float32, name=f"p2{j}_{t}", tag="p2")
                    for k in range(KI // 2):
                        lhs = h1v[:, 2 * k : 2 * k + 2, t * 128 : (t + 1) * 128]
                        nc.tensor.matmul(
                            p2[:, 0:512],
                            lhs,
                            w2v[:, 2 * k : 2 * k + 2, 0:512],
                            start=(k == 0),
                            stop=(k == KI // 2 - 1),
                            perf_mode=DR,
                        )
                        nc.tensor.matmul(
                            p2[:, 512:768],
                            lhs,
                            w2v[:, 2 * k : 2 * k + 2, 512:768],
                            start=(k == 0),
                            stop=(k == KI // 2 - 1),
                            perf_mode=DR,
                        )
                    # weighted accumulate into acc (wsel carries the 1/SW2):
                    # ScalarE does the p_sel multiply, DVE the accumulate
                    wcol = wsel[:, t * G + j : t * G + j + 1]
                    # combine runs fully on DVE: ScalarE stays free for the
                    # gelu ACTs, which otherwise head-of-line-block the psum
                    # drain at expert boundaries
                    if j == 0 and not with_b2:
                        nc.vector.tensor_scalar_mul(
                            acc[:, t * H : (t + 1) * H], p2[:, 0:768], wcol[:]
                        )
                    elif j < G - 1:
                        tmpc = tmpp.tile(
                            [128, H], dt.float32, name=f"tmpc{j}_{t}", tag="tmpc"
                        )
                        nc.vector.tensor_scalar_mul(tmpc[:], p2[:, 0:768], wcol[:])
                        nc.vector.tensor_add(
                            acc[:, t * H : (t + 1) * H],
                            acc[:, t * H : (t + 1) * H],
                            tmpc[:],
                        )
                    else:
                        # final expert: both p2-reading muls first (frees the
                        # psum bank for t+2 asap), then the adds + bf16 DMAs
                        tcs = []
                        for c0, c1 in ((0, 512), (512, 768)):
                            tmpc = tmpp.tile(
                                [128, c1 - c0],
                                dt.float32,
                                name=f"tmpc{j}_{t}_{c0}",
                                tag=f"tmpch{c0}",
                            )
                            nc.vector.tensor_scalar_mul(tmpc[:], p2[:, c0:c1], wcol[:])
                            tcs.append((c0, c1, tmpc))
                        for c0, c1, tmpc in tcs:
                            nc.vector.tensor_add(
                                accb[:, t * H + c0 : t * H + c1],
                                acc[:, t * H + c0 : t * H + c1],
                                tmpc[:],
                            )
                            nc.sync.dma_start(
                                out=out_d.ap()[t * 128 : (t + 1) * 128, c0:c1],
                                in_=accb[:, t * H + c0 : t * H + c1],
                            )

            with tc.tile_pool(name="ps1", bufs=2, space="PSUM") as ps1:
                h1g0 = h1gp.tile([128, MI * B], dt.float8e4, name="h1g0", tag="h1g")
                expT = routp.tile([NR, B], dt.float32, name="expTt")

                with (
                    tc.tile_pool(name="psT", bufs=1, space="PSUM") as psTp,
                    tc.tile_pool(name="psm", bufs=2, space="PSUM") as psmp,
                ):
                    psT = psTp.tile([NR, B], dt.float32, name="psTt")
                    # PE p-state warmup while the input DMAs stream
                    for w in range(2):
                        nc.tensor.matmul(
                            psT[0:32, 0:512],
                            ident[:],
                            wrhs[:],
                            start=True,
                            stop=True,
                        )

                    # fc1(0) m0-3 leads (first w1 piece); routing matmuls slot
                    # in behind, then the rest of fc1(0)
                    emit_fc1(0, w1v0, b1t0, h1g0, ps1, range(0, MPP))

                    # routing: logitsT[NR, B] in fp8 DoubleRow, scale absorbed
                    # by the exp() activation
                    for k in range(KH // 2):
                        for n in range(2):
                            nc.tensor.matmul(
                                psT[:, n * 512 : (n + 1) * 512],
                                rfv[:, 2 * k : 2 * k + 2, :],
                                x8v[:, 2 * k : 2 * k + 2, n * 512 : (n + 1) * 512],
                                start=(k == 0),
                                stop=False,
                                perf_mode=DR,
                            )
                    for k in range(KH // 2):
                        for n in range(2):
                            nc.tensor.matmul(
                                psT[:, n * 512 : (n + 1) * 512],
                                rav[:, 2 * k : 2 * k + 2, :],
                                xa8v[:, 2 * k : 2 * k + 2, n * 512 : (n + 1) * 512],
                                start=False,
                                stop=(k == KH // 2 - 1),
                                perf_mode=DR,
                            )
                    # exp(logits + bias) in one ACT op (small logits: no
                    # max-subtract); 1/2048 descale folded into the ACT
                    nc.scalar.activation(
                        expT[:], psT[:], AF.Exp, bias=bcatT[:], scale=1.0 / (SX * SRF)
                    )

                    emit_fc1(0, w1v0, b1t0, h1g0, ps1, range(MPP, MI))

                    # transpose expT to token-major (one psum tile), then a
                    # batched DVE softmax tail over all 8 token chunks
                    trp8 = psmp.tile([128, MB * NR], dt.float32, name="trp8", tag="trp")
                    for m in range(MB):
                        nc.tensor.transpose(
                            trp8[:, m * NR : (m + 1) * NR],
                            expT[:, m * 128 : (m + 1) * 128],
                            ident[0:NR, 0:NR],
                        )
                    t8 = trp8[:].rearrange("p (m n) -> p m n", n=NR)
                    sg8 = smp.tile([128, MB], dt.float32, name="sg8")
                    nc.vector.reduce_sum(sg8[:], t8[:, :, 24:27], axis=mybir.AxisListType.X)
                    rg8 = smp.tile([128, MB], dt.float32, name="rg8")
                    nc.vector.reciprocal(rg8[:], sg8[:])
                    pgn8 = smp.tile([128, MB * G], dt.float32, name="pgn8")
                    pgn8v = pgn8[:].rearrange("p (m g) -> p m g", g=G)
                    nc.vector.tensor_mul(
                        pgn8v,
                        t8[:, :, 24:27],
                        rg8[:].rearrange("p m -> p m ()").broadcast_to([128, MB, G]),
                    )
                    se8 = smp.tile([128, MB * G], dt.float32, name="se8")
                    se8v = se8[:].rearrange("p (m g) -> p m g", g=G)
                    nc.vector.reduce_sum(
                        se8v,
                        t8[:, :, 0:24].rearrange("p m (g e) -> p m g e", e=E),
                        axis=mybir.AxisListType.X,
                    )
                    re8 = smp.tile([128, MB * G], dt.float32, name="re8")
                    nc.vector.reciprocal(re8[:], se8[:])
                    pe08 = smp.tile([128, MB * G], dt.float32, name="pe08")
                    nc.vector.tensor_mul(
                        pe08[:].rearrange("p (m g) -> p m g", g=G),
                        t8[:, :, 0:24].rearrange("p m (g e) -> p m g e", e=E)[:, :, :, 0],
                        re8[:].rearrange("p (m g) -> p m g", g=G),
                    )
                    # wsel = p_group * p_expert0 / SW2 (fc2 descale folded in)
                    nc.vector.tensor_mul(wsel[:], pe08[:], pgn8[:])
                    nc.vector.tensor_scalar_mul(wsel[:], wsel[:], 1.0 / SW2)

                    for m in range(MB):
                        if with_b2:
                            # acc[t] bias init: sum_j wsel[t,j]*(SW2*b2[j,:])
                            bt = tmpp.tile(
                                [128, H], dt.float32, name=f"bt{m}", tag="tmpc"
                            )
                            nc.vector.tensor_scalar_mul(
                                acc[:, m * H : (m + 1) * H],
                                b2repl[:, 0:H],
                                wsel[:, m * G : m * G + 1],
                            )
                            for j in range(1, G):
                                nc.vector.tensor_scalar_mul(
                                    bt[:],
                                    b2repl[:, j * H : (j + 1) * H],
                                    wsel[:, m * G + j : m * G + j + 1],
                                )
                                nc.vector.tensor_add(
                                    acc[:, m * H : (m + 1) * H],
                                    acc[:, m * H : (m + 1) * H],
                                    bt[:],
                                )

                # ---- fc2(0) + experts 1,2 ----
                with tc.tile_pool(name="ps2", bufs=2, space="PSUM") as ps2:
                    emit_fc2(0, w2t0, h1g0, ps2)
                    for j in range(1, G):
                        w1t = wp.tile([128, W1COLS], dt.float8e4, name=f"w1t{j}", tag="w1")
                        nc.sync.dma_start(out=w1t[:], in_=w1_d.ap()[j])
                        b1t = b1p.tile([128, MI], dt.float32, name=f"b1t{j}", tag="b1")
                        nc.sync.dma_start(out=b1t[:], in_=b1_d.ap()[j])
                        w2t = wp.tile([128, KI * H], dt.float8e4, name=f"w2t{j}", tag="w2")
                        nc.sync.dma_start(out=w2t[:], in_=w2_d.ap()[j])
                        h1g = h1gp.tile(
                            [128, MI * B], dt.float8e4, name=f"h1g{j}", tag="h1g"
                        )
                        w1v = w1t[:].rearrange("p (m k i) -> p m k i", k=KH, i=128)
                        # m22/23 first: fc2 t0's k=11 needs their gelu output,
                        # so get their ACTs off the critical path early
                        emit_fc1(j, w1v, b1t, h1g, ps1, [MI - 2, MI - 1] + list(range(MI - 2)))
                        emit_fc2(j, w2t, h1g, ps2)

    nc.compile()
    return nc


def _get_nc(with_b2=False):
    if with_b2 not in _nc_cache:
        _nc_cache[with_b2] = _build_nc(with_b2)
    return _nc_cache[with_b2]


def _prepare(inputs):
    h_fused = np.asarray(inputs["h_fused"], np.float32)
    h_aspect = np.asarray(inputs["h_aspect"], np.float32)
    Wc = np.asarray(inputs["Wc"], np.float32)
    bc = np.asarray(inputs["bc"], np.float32)
    Wg = np.asarray(inputs["Wg"], np.float32)
    bg = np.asarray(inputs["bg"], np.float32)
    Wr = np.asarray(inputs["Wr"], np.float32)
    br = np.asarray(inputs["br"], np.float32)
    W1 = np.asarray(inputs["W1"], np.float32)
    b1 = np.asarray(inputs["b1"], np.float32)
    W2 = np.asarray(inputs["W2"], np.float32)
    b2 = np.asarray(inputs["b2"], np.float32)

    with_b2 = bool(np.any(b2))

    def q8(x, s):
        return np.clip(np.asarray(x, np.float64) * s, -240.0, 240.0).astype(FP8)

    # fold cond_proj through the expert routers (float64 for the fold)
    Wcr = np.einsum("ch,ghe->cge", Wc.astype(np.float64), Wr.astype(np.float64))
    bcr = np.einsum("h,ghe->ge", bc.astype(np.float64), Wr.astype(np.float64)) + br

    KH, KI, MI = H // 128, I // 128, I // 128

    def pack_pk(a, ncols):
        # [K*128, ncols] -> [128, K*ncols] partition-major
        k = a.shape[0] // 128
        return np.ascontiguousarray(
            a.reshape(k, 128, ncols).transpose(1, 0, 2).reshape(128, k * ncols)
        )

    xt8 = pack_pk(q8(h_fused.T, SX), B)
    xa8 = pack_pk(q8(h_aspect.T, SX), B)

    in_maps = []
    for c in range(NCORES):
        perm = [c] + [e for e in range(E) if e != c]
        Wcr_p = Wcr[:, :, perm]  # [2H, G, E]
        bcr_p = np.asarray(bcr, np.float64)[:, perm]  # [G, E]
        rf = np.concatenate(
            [Wcr_p[:H].reshape(H, G * E), Wg.astype(np.float64),
             np.zeros((H, 5), np.float64)], axis=1
        )
        ra = np.concatenate(
            [Wcr_p[H:].reshape(H, G * E), np.zeros((H, G + 5), np.float64)], axis=1
        )
        bcat = np.concatenate(
            [bcr_p.reshape(G * E), bg.astype(np.float64), np.zeros(5)]
        )[:, None].astype(np.float32)
        # w1[g]: [H, I] -> [128, (m k i128)] m-major; w2[g]: [I, H] -> [128, (k h)]
        w1p = np.stack(
            [
                q8(W1[g, c], SW1)
                .reshape(KH, 128, MI, 128)
                .transpose(1, 2, 0, 3)
                .reshape(128, MI * KH * 128)
                for g in range(G)
            ]
        )
        w2p = np.stack([pack_pk(q8(W2[g, c], SW2), H) for g in range(G)])
        b1p_ = np.stack(
            [np.ascontiguousarray(b1[g, c].reshape(MI, 128).T) for g in range(G)]
        )
        im = {
            "xt8": xt8,
            "xa8": xa8,
            "rf": pack_pk(q8(rf, SRF), 32),
            "ra": pack_pk(q8(ra, SRF), 32),
            "bcat": np.ascontiguousarray(bcat),
            "w1": np.ascontiguousarray(w1p),
            "b1": np.ascontiguousarray(b1p_),
            "w2": np.ascontiguousarray(w2p),
        }
        if with_b2:
            im["b2"] = (np.ascontiguousarray(b2[:, c]) * SW2).astype(BF16)
        in_maps.append(im)

    return h_fused, in_maps, with_b2


def kernel(**inputs):
    from concourse.bass_utils import run_bass_kernel_spmd

    h_fused, in_maps, with_b2 = _prepare(inputs)
    nc = _get_nc(with_b2)
    res = run_bass_kernel_spmd(nc, in_maps, core_ids=list(range(NCORES)))
    out = h_fused.copy()
    for c in range(NCORES):
        out += np.asarray(res.results[c]["out"], np.float32)
    return out


def run_traced(**inputs):
    """Profiled run: returns BassKernelResults with exec_time_ns."""
    from concourse.bass_utils import run_bass_kernel_spmd

    h_fused, in_maps, with_b2 = _prepare(inputs)
    nc = _get_nc(with_b2)
    res = run_bass_kernel_spmd(nc, in_maps, core_ids=list(range(NCORES)), trace=True)
    return res
